# revision 2
# baseline (speedup 1.0000x reference)
"""Trainium2 Bass kernel for nn_BoothGroupQuant.

Booth/NAF group quantization: q = rne(x*128); NAF-decompose each q into
signed power-of-two digits; per group of 16 consecutive elements keep only
the 8 largest-exponent digits (ties: lower exponent-position first by
element order); reconstruct and scale by 1/128.

Core identity: with t = 3q, u = t ^ q, the NAF nonzero-digit mask of q is u
(digit at exponent e <-> bit e+1), with positive digits at u & t and negative
at u & q -- valid directly on two's-complement negatives.  Per-group top-8
selection is done with int16 SWAR band counters (4 bands of 3 exponents),
two grouped reduces, one segmented scan for in-band ranks, and a packed
guard-bit compare.  Design range |q| <= 2730 (actual data max 668).

Sharding: embarrassingly parallel -- the flat element axis is split into 8
contiguous 524288-element shards, one per NeuronCore (groups of 16 never
straddle a shard boundary).
"""
import os
import sys

import numpy as np

for _p in ("/opt/trn_rl_repo", "/root/.axon_site/_ro/trn_rl_repo"):
    if os.path.isdir(_p) and _p not in sys.path:
        sys.path.insert(0, _p)

import concourse.bacc as bacc
import concourse.mybir as mybir
from concourse import bass_utils
from concourse.tile import TileContext

N_CORES = 8
FULL_SHAPE = (4, 1024, 32, 32)
N_TOTAL = 4 * 1024 * 32 * 32          # 4194304
N_CORE = N_TOTAL // N_CORES           # 524288
P = 128                               # SBUF partitions
F_TOTAL = N_CORE // P                 # 4096 free elems per partition
F_CHUNK = 1024                        # free elems per chunk
N_CHUNKS = F_TOTAL // F_CHUNK
SF = 0.0078125

i16 = mybir.dt.int16
f32 = mybir.dt.float32
Alu = mybir.AluOpType
Act = mybir.ActivationFunctionType
AX = mybir.AxisListType

_CACHE = {}


def _build():
    nc = bacc.Bacc("TRN2")
    x_in = nc.dram_tensor("x", [P, F_TOTAL], f32, kind="ExternalInput")
    y_out = nc.dram_tensor("y", [P, F_TOTAL], f32, kind="ExternalOutput")

    with TileContext(nc) as tc:
        with tc.tile_pool(name="const", bufs=1) as cpool:
            # segment mask: 0 at each group start, 1 elsewhere
            seg = cpool.tile([P, F_CHUNK], i16)
            nc.vector.memset(seg, 1)
            nc.vector.memset(
                seg.rearrange("p (g s) -> p g s", s=16)[:, :, 0:1], 0)

            with tc.tile_pool(name="work", bufs=2) as pool:
                for ci in range(N_CHUNKS):
                    _chunk(nc, pool, seg, x_in, y_out, ci)

    nc.compile()
    return nc


def _chunk(nc, pool, seg, x_in, y_out, ci):
    Fc = F_CHUNK
    Gc = Fc // 16
    sl = slice(ci * Fc, (ci + 1) * Fc)

    def grp(ap):
        return ap.rearrange("p (g s) -> p g s", s=16)

    def bc(tiny):
        # [P, Gc] -> broadcast over the 16 group elements
        return tiny[:, :, None].broadcast_to((P, Gc, 16))

    xt = pool.tile([P, Fc], f32)
    nc.sync.dma_start(out=xt, in_=x_in[:, sl])

    # q = rne(x*128) int16   (|q| <= 668 on this input; no clip needed)
    q = pool.tile([P, Fc], i16)
    nc.scalar.activation(q, xt, Act.Copy, scale=128.0)

    # t = 3q ; u = t ^ q  (NAF mask, bits 1..12)
    t = pool.tile([P, Fc], i16)
    nc.vector.tensor_scalar(t, q, 3, None, Alu.mult)
    u = pool.tile([P, Fc], i16)
    nc.vector.tensor_tensor(u, t, q, Alu.bitwise_xor)

    # band popcounts: c fields {0,3,6,9} = per-element band counts (0..3)
    a1 = pool.tile([P, Fc], i16)
    nc.vector.tensor_scalar(a1, u, 1, 0x249, Alu.logical_shift_right,
                            Alu.bitwise_and)
    a2 = pool.tile([P, Fc], i16)
    nc.vector.tensor_scalar(a2, u, 2, 0x249, Alu.logical_shift_right,
                            Alu.bitwise_and)
    a3 = pool.tile([P, Fc], i16)
    nc.vector.tensor_scalar(a3, u, 3, 0x249, Alu.logical_shift_right,
                            Alu.bitwise_and)
    c12 = pool.tile([P, Fc], i16)
    nc.vector.tensor_tensor(c12, a1, a2, Alu.add)
    c = pool.tile([P, Fc], i16)
    nc.vector.tensor_tensor(c, c12, a3, Alu.add)
    ce = pool.tile([P, Fc], i16)
    nc.vector.tensor_scalar(ce, c, 0x1C7, None, Alu.bitwise_and)
    co = pool.tile([P, Fc], i16)
    nc.vector.tensor_scalar(co, c, 3, 0x1C7, Alu.logical_shift_right,
                            Alu.bitwise_and)

    # group band totals (fields 0-5, 6-11; sums <= 48)
    RE = pool.tile([P, Gc], i16)
    RO = pool.tile([P, Gc], i16)
    with nc.allow_low_precision(reason="exact small int sums"):
        nc.vector.tensor_reduce(RE, grp(ce), AX.X, Alu.add)
        nc.vector.tensor_reduce(RO, grp(co), AX.X, Alu.add)

    # tiny-domain: band sums, crossing band b*, theta
    B0 = pool.tile([P, Gc], i16)
    nc.vector.tensor_scalar(B0, RE, 63, None, Alu.bitwise_and)
    B2 = pool.tile([P, Gc], i16)
    nc.vector.tensor_scalar(B2, RE, 6, 63, Alu.logical_shift_right,
                            Alu.bitwise_and)
    B1 = pool.tile([P, Gc], i16)
    nc.vector.tensor_scalar(B1, RO, 63, None, Alu.bitwise_and)
    B3 = pool.tile([P, Gc], i16)
    nc.vector.tensor_scalar(B3, RO, 6, 63, Alu.logical_shift_right,
                            Alu.bitwise_and)
    s2 = pool.tile([P, Gc], i16)
    nc.vector.tensor_tensor(s2, B3, B2, Alu.add)
    s1 = pool.tile([P, Gc], i16)
    nc.vector.tensor_tensor(s1, s2, B1, Alu.add)
    g3 = pool.tile([P, Gc], i16)
    nc.vector.tensor_scalar(g3, B3, 8, None, Alu.is_ge)
    g2 = pool.tile([P, Gc], i16)
    nc.vector.tensor_scalar(g2, s2, 8, None, Alu.is_ge)
    g1 = pool.tile([P, Gc], i16)
    nc.vector.tensor_scalar(g1, s1, 8, None, Alu.is_ge)
    bsum = pool.tile([P, Gc], i16)
    nc.vector.tensor_tensor(bsum, g3, g2, Alu.add)
    bstar = pool.tile([P, Gc], i16)
    nc.vector.tensor_tensor(bstar, bsum, g1, Alu.add)
    amt = pool.tile([P, Gc], i16)
    nc.vector.tensor_scalar(amt, bstar, 3, 1, Alu.mult, Alu.add)

    # Cab = B3*(1-g3) + B2*(1-g2) + B1*(1-g1);  theta = 8 - Cab in [1, 8]
    ng3 = pool.tile([P, Gc], i16)
    nc.vector.tensor_scalar(ng3, g3, -1, 1, Alu.mult, Alu.add)
    ng2 = pool.tile([P, Gc], i16)
    nc.vector.tensor_scalar(ng2, g2, -1, 1, Alu.mult, Alu.add)
    ng1 = pool.tile([P, Gc], i16)
    nc.vector.tensor_scalar(ng1, g1, -1, 1, Alu.mult, Alu.add)
    m3 = pool.tile([P, Gc], i16)
    nc.vector.tensor_tensor(m3, B3, ng3, Alu.mult)
    m2 = pool.tile([P, Gc], i16)
    nc.vector.tensor_tensor(m2, B2, ng2, Alu.mult)
    m1 = pool.tile([P, Gc], i16)
    nc.vector.tensor_tensor(m1, B1, ng1, Alu.mult)
    m32 = pool.tile([P, Gc], i16)
    nc.vector.tensor_tensor(m32, m3, m2, Alu.add)
    Cab = pool.tile([P, Gc], i16)
    nc.vector.tensor_tensor(Cab, m32, m1, Alu.add)
    theta = pool.tile([P, Gc], i16)
    nc.vector.tensor_scalar(theta, Cab, -1, 8, Alu.mult, Alu.add)

    # stage-2 per-element: band digits, in-band per-exponent ranks
    w = pool.tile([P, Fc], i16)
    nc.vector.tensor_tensor(grp(w), grp(u), bc(amt), Alu.logical_shift_right)
    v = pool.tile([P, Fc], i16)
    nc.vector.tensor_scalar(v, w, 7, None, Alu.bitwise_and)
    sm = pool.tile([P, Fc], i16)
    nc.vector.tensor_scalar(sm, v, 0x111, None, Alu.mult)
    s = pool.tile([P, Fc], i16)
    nc.vector.tensor_scalar(s, sm, 0x421, None, Alu.bitwise_and)
    Pm = pool.tile([P, Fc], i16)
    nc.vector.tensor_tensor_scan(Pm, seg, s, 0.0, Alu.mult, Alu.add)

    # tiny: per-exp thresholds packed with guard bits
    TP = pool.tile([P, Gc], i16)
    nc.vector.tensor_copy(TP, grp(Pm)[:, :, 15])
    n2 = pool.tile([P, Gc], i16)
    nc.vector.tensor_scalar(n2, TP, 10, 31, Alu.logical_shift_right,
                            Alu.bitwise_and)
    n1 = pool.tile([P, Gc], i16)
    nc.vector.tensor_scalar(n1, TP, 5, 31, Alu.logical_shift_right,
                            Alu.bitwise_and)
    th1 = pool.tile([P, Gc], i16)
    nc.vector.tensor_tensor(th1, theta, n2, Alu.subtract)
    th0 = pool.tile([P, Gc], i16)
    nc.vector.tensor_tensor(th0, th1, n1, Alu.subtract)
    th1c = pool.tile([P, Gc], i16)
    nc.vector.tensor_scalar(th1c, th1, 0, None, Alu.max)
    th0c = pool.tile([P, Gc], i16)
    nc.vector.tensor_scalar(th0c, th0, 0, None, Alu.max)
    t1s = pool.tile([P, Gc], i16)
    nc.vector.tensor_scalar(t1s, th1c, 32, None, Alu.mult)
    t2s = pool.tile([P, Gc], i16)
    nc.vector.tensor_scalar(t2s, theta, 1024, None, Alu.mult)
    tha = pool.tile([P, Gc], i16)
    nc.vector.tensor_tensor(tha, th0c, t1s, Alu.add)
    thb = pool.tile([P, Gc], i16)
    nc.vector.tensor_tensor(thb, tha, t2s, Alu.add)
    ThGp = pool.tile([P, Gc], i16)
    nc.vector.tensor_scalar(ThGp, thb, 0x3DEF, None, Alu.add)

    # per-element packed compare: guard bit j <=> excl_rank_j < theta_j
    Y = pool.tile([P, Fc], i16)
    nc.vector.tensor_tensor(Y, Pm, s, Alu.subtract)
    X = pool.tile([P, Fc], i16)
    nc.vector.tensor_tensor(grp(X), bc(ThGp), grp(Y), Alu.subtract)
    # gather guard bits {4,9,14} -> {0,1,2} (int16-safe two-mult form)
    K3hi = pool.tile([P, Fc], i16)
    nc.vector.tensor_scalar(K3hi, X, 12, 4, Alu.logical_shift_right,
                            Alu.bitwise_and)
    Y2lo = pool.tile([P, Fc], i16)
    nc.vector.tensor_scalar(Y2lo, X, 4, 0x21, Alu.logical_shift_right,
                            Alu.bitwise_and)
    K3m = pool.tile([P, Fc], i16)
    nc.vector.tensor_scalar(K3m, Y2lo, 0x11, None, Alu.mult)
    K3lo = pool.tile([P, Fc], i16)
    nc.vector.tensor_scalar(K3lo, K3m, 4, 3, Alu.logical_shift_right,
                            Alu.bitwise_and)
    K3 = pool.tile([P, Fc], i16)
    nc.vector.tensor_tensor(K3, K3lo, K3hi, Alu.bitwise_or)
    kb = pool.tile([P, Fc], i16)
    nc.vector.tensor_tensor(kb, w, K3, Alu.bitwise_and)
    Kband = pool.tile([P, Fc], i16)
    nc.vector.tensor_scalar(Kband, kb, -8, None, Alu.bitwise_or)
    wk = pool.tile([P, Fc], i16)
    nc.vector.tensor_tensor(wk, w, Kband, Alu.bitwise_and)
    UK = pool.tile([P, Fc], i16)
    nc.vector.tensor_tensor(grp(UK), grp(wk), bc(amt), Alu.logical_shift_left)

    pos = pool.tile([P, Fc], i16)
    nc.vector.tensor_tensor(pos, UK, t, Alu.bitwise_and)
    neg = pool.tile([P, Fc], i16)
    nc.vector.tensor_tensor(neg, UK, q, Alu.bitwise_and)
    val = pool.tile([P, Fc], i16)
    nc.vector.tensor_tensor(val, pos, neg, Alu.subtract)

    yt = pool.tile([P, Fc], f32)
    nc.scalar.activation(yt, val, Act.Copy, scale=SF / 2.0)
    nc.sync.dma_start(out=y_out[:, sl], in_=yt)


def _get_nc():
    if "nc" not in _CACHE:
        _CACHE["nc"] = _build()
    return _CACHE["nc"]


def kernel(x: np.ndarray, _trace: bool = False, _trace_kwargs=None):
    assert x.shape == FULL_SHAPE and x.dtype == np.float32, (x.shape, x.dtype)
    nc = _get_nc()
    flat = np.ascontiguousarray(x).reshape(N_CORES, P, F_TOTAL)
    in_maps = [{"x": flat[i]} for i in range(N_CORES)]
    kw = {}
    if _trace:
        kw = {"trace": True, **(_trace_kwargs or {})}
    res = bass_utils.run_bass_kernel_spmd(
        nc, in_maps, core_ids=list(range(N_CORES)), **kw)
    out = np.stack([res.results[i]["y"] for i in range(N_CORES)], axis=0)
    out = out.reshape(FULL_SHAPE).astype(np.float32)
    if _trace:
        return out, res
    return out
